# revision 34
# baseline (speedup 1.0000x reference)
"""BEiT-style windowed attention with relative position bias, on 8 trn2 cores.

Sharding: data-parallel over batch (32 batches -> 4 per core). Weights, the
gathered relative-position bias table, and the identity matrix are replicated.

IO is minimized because per-exec input staging dominates wall time on this
dispatch path: all bf16 operands ship in ONE packed DRAM buffer (xt | wqkv |
wp | ident | proj-bias as exact bf16 hi/lo pair), the host-gathered rel-pos
bias ships as fp8 e4m3 (|bias| ~ 0.02 inside softmax logits, so 3% relative
quantization error is negligible), and the output returns as fp16.

Device pipeline per core (all matmuls bf16 in / f32 accumulate):
  Phase 1: batch-major. Per batch block: qkT chunks for all 12 m-tiles
           (512+65 column groups — a matmul may not cross a PSUM bank), then
           v tiles with an appended ones column per head (softmax
           denominators). x and the qk weights are packed so the first
           matmul waits only on one small weight tile + one batch block.
  Phase 2: per head: DMA the fp8 bias image, upcast to bf16 once on DVE (so
           the PE never switches dtype). Per (head, batch): scores^T = bias
           via identity-matmul accumulation + QK matmuls -> exp on ScalarE
           -> PV matmul with [v|1] stationary -> denominators on PSUM row 64
           -> row-64->0 DMA, partition_broadcast, reciprocal, normalize.
  Phase 3: out^T = Wp^T.T @ O^T + b -> fp16 DMA out, host casts + transposes.

Layouts are host-prepped into SBUF-image form [128, n*cols] so every DMA is a
straight contiguous copy.
"""

import numpy as np
import ml_dtypes

import concourse.bass as bass
import concourse.tile as tile
from concourse import bacc, mybir
from concourse.bass_utils import run_bass_kernel_spmd

BF16 = mybir.dt.bfloat16
FP8 = mybir.dt.float8e4
F16 = mybir.dt.float16
F32 = mybir.dt.float32
AF = mybir.ActivationFunctionType

NCORES = 8
B = 32
BPC = B // NCORES          # batches per core
N = 577                    # sequence length
C = 768
H = 12
HD = 64
R = BPC * N                # rows per core (2308)
CT = C // 128              # 6 contraction tiles
MT = 12                    # qk output row-tiles (1536/128)
JTS = [128, 128, 128, 128, 65]   # j tiles of N
ECOLS = 2920               # packed score/E columns: 5*512 + 5*72
RCHUNKS = [(0, 512), (512, 512), (1024, 512), (1536, 512), (2048, 260)]

# mega buffer column offsets (bf16 image [128, MEGA_COLS]); the rel-pos bias
# rides along as raw fp8 bytes (2 per bf16 col) — finite e4m3 byte pairs can
# never alias a bf16 NaN/Inf pattern (that needs a 0x7F/0xFF high byte, which
# is non-finite in e4m3), so the bf16-typed transport is safe.
# xt is packed batch-major ([b, kk, c] per partition) and the qk weights
# m-major so phase-1 matmuls can start as soon as the first small DMA lands.
XBB = CT * N                        # 3462 cols per batch block
XT_OFF = 0
WQK_OFF = XT_OFF + BPC * XBB        # 13848
WV_OFF = WQK_OFF + MT * CT * 128    # 23064
WP_OFF = WV_OFF + CT * C            # 27672
ID_OFF = WP_OFF + CT * C            # 32280
PBH_OFF = ID_OFF + 128              # 32408
PBL_OFF = PBH_OFF + CT              # 32414
BIAS_OFF = PBL_OFF + CT             # 32420
BIAS_HCOLS = ECOLS // 2             # 1460 bf16 cols per head
MEGA_COLS = BIAS_OFF + H * BIAS_HCOLS  # 49940

_PROGRAM = None


def build_program(reps=1):
    """reps > 1 unrolls the whole kernel body that many times in one NEFF —
    used only by test.py to measure per-execution device time as
    (T_reps - T_1) / (reps - 1), cancelling dispatch/staging costs."""
    nc = bacc.Bacc(trn_type="TRN2", name="beit_attn")

    mega_d = nc.dram_tensor("mega", [128, MEGA_COLS], BF16, kind="ExternalInput")
    out_d = nc.dram_tensor("ftout", [128, CT * R], F16, kind="ExternalOutput")

    def bias_src(h):
        lo = BIAS_OFF + h * BIAS_HCOLS
        return mega_d[:, lo : lo + BIAS_HCOLS].bitcast(FP8)

    with tile.TileContext(nc) as tc:
        for _rep in range(reps):
            _build_body(nc, tc, mega_d, out_d, bias_src)

    nc.compile()
    return nc


def _build_body(nc, tc, mega_d, out_d, bias_src):
    with (
            tc.tile_pool(name="static", bufs=1) as sp,
            tc.tile_pool(name="qk", bufs=1) as qk_pool,
            tc.tile_pool(name="v1", bufs=1) as v1_pool,
        ):
            ident = sp.tile([128, 128], BF16, tag="ident")
            nc.sync.dma_start(ident[:], mega_d[:, ID_OFF : ID_OFF + 128])
            pbhl = sp.tile([128, 2 * CT], BF16, tag="pbhl")
            nc.sync.dma_start(pbhl[:], mega_d[:, PBH_OFF : PBH_OFF + 2 * CT])
            pb = sp.tile([128, CT], F32, tag="pb")
            nc.vector.tensor_add(pb[:, :], pbhl[:, 0:CT], pbhl[:, CT : 2 * CT])
            wp_sb = sp.tile([128, CT * C], BF16, tag="wp")
            nc.sync.dma_start(wp_sb[:], mega_d[:, WP_OFF : WP_OFF + CT * C])
            ot_sb = sp.tile([128, CT * R], BF16, tag="ot")
            # First head's bias staged from the long-lived pool so the first
            # score matmuls don't wait on the phase-1 pool-release barrier.
            bias0_sb = sp.tile([128, ECOLS], FP8, tag="bias0")
            nc.sync.dma_start(bias0_sb[:], bias_src(0))

            qk_t = [qk_pool.tile([128, R], BF16, tag=f"qk{m}", name=f"qk{m}") for m in range(MT)]
            v1_t = [
                [v1_pool.tile([128, 780], BF16, tag=f"v1_{b}_{t}", name=f"v1_{b}_{t}") for t in range(5)]
                for b in range(BPC)
            ]

            # ---------------- Phase 1: QKV projections -------------------
            # Batch-major: all qkT m-tiles' batch-b chunks, then v(b), so the
            # first matmul only waits for one small weight tile + one batch
            # block of x, and phase 2's (h, b=0) operands finish earliest.
            with (
                tc.tile_pool(name="ph1", bufs=1) as p1,
                tc.tile_pool(name="psq", bufs=3, space="PSUM") as psq,
                tc.tile_pool(name="psv", bufs=2, space="PSUM") as psv,
            ):
                wqk_m = []
                for m in range(MT):
                    w = p1.tile([128, CT * 128], BF16, tag=f"wqk{m}")
                    nc.sync.dma_start(
                        w[:], mega_d[:, WQK_OFF + 768 * m : WQK_OFF + 768 * (m + 1)]
                    )
                    wqk_m.append(w)
                xt_b = []
                for b in range(BPC):
                    xb = p1.tile([128, XBB], BF16, tag=f"xtb{b}")
                    nc.sync.dma_start(
                        xb[:], mega_d[:, XT_OFF + XBB * b : XT_OFF + XBB * (b + 1)]
                    )
                    xt_b.append(xb)
                wv_sb = p1.tile([128, CT * C], BF16, tag="wv")
                nc.sync.dma_start(wv_sb[:], mega_d[:, WV_OFF : WV_OFF + CT * C])

                def emit_qkT(m, b):
                    # 512 + 65 column groups: a matmul output may not cross a
                    # PSUM bank boundary (2 KB = 512 f32).
                    ps = psq.tile([128, 1024], F32, tag="psq")
                    for c0, cn in ((0, 512), (512, 65)):
                        for kk in range(CT):
                            nc.tensor.matmul(
                                ps[:, c0 : c0 + cn],
                                wqk_m[m][:, 128 * kk : 128 * (kk + 1)],
                                xt_b[b][:, N * kk + c0 : N * kk + c0 + cn],
                                start=(kk == 0),
                                stop=(kk == CT - 1),
                            )
                    if (m + b) % 2 == 0:
                        nc.vector.tensor_copy(qk_t[m][:, N * b : N * (b + 1)], ps[:, 0:N])
                    else:
                        nc.scalar.copy(qk_t[m][:, N * b : N * (b + 1)], ps[:, 0:N])

                def emit_v(b):
                    for t in range(5):
                        kj = JTS[t]
                        for vc0, vcn, h0 in ((0, 512, 0), (512, 256, 8)):
                            ps = psv.tile([128, 512], F32, tag="psv")
                            for kk in range(CT):
                                nc.tensor.matmul(
                                    ps[:kj, :vcn],
                                    xt_b[b][:, N * kk + 128 * t : N * kk + 128 * t + kj],
                                    wv_sb[:, C * kk + vc0 : C * kk + vc0 + vcn],
                                    start=(kk == 0),
                                    stop=(kk == CT - 1),
                                )
                            dest = v1_t[b][t][:kj, 65 * h0 : 65 * (h0 + vcn // 64)]
                            dest = dest.rearrange("p (h d) -> p h d", d=65)[:, :, 0:64]
                            src = ps[:kj, :vcn].rearrange("p (h d) -> p h d", d=64)
                            nc.vector.tensor_copy(dest, src)
                        ones_ap = v1_t[b][t][:, :].rearrange("p (h d) -> p h d", d=65)[
                            :, :, 64:65
                        ]
                        nc.gpsimd.memset(ones_ap, 1.0)

                for b in range(BPC):
                    for m in range(MT):
                        emit_qkT(m, b)
                    emit_v(b)

            # ---------------- Phase 2: attention -------------------------
            with (
                tc.tile_pool(name="ph2", bufs=2) as p2,
                tc.tile_pool(name="score", bufs=1, space="PSUM") as score_pool,
                tc.tile_pool(name="pvp", bufs=1, space="PSUM") as pv_pool,
            ):
                for h in range(H):
                    # Bias ships as fp8 but is upcast to bf16 once per head on
                    # DVE so the PE never switches dtype between the QK and
                    # bias-injection matmuls.
                    if h == 0:
                        bias8 = bias0_sb
                    else:
                        bias8 = p2.tile([128, ECOLS], FP8, tag="bias8")
                        nc.sync.dma_start(bias8[:], bias_src(h))
                    bias_t = p2.tile([128, ECOLS], BF16, tag="bias")
                    nc.vector.tensor_copy(bias_t[:, :], bias8[:, :])
                    qp = 64 * (h % 2)
                    qm, km = h // 2, 6 + h // 2
                    for b in range(BPC):
                        T0 = score_pool.tile([128, 1024], F32, tag="T0")
                        T1 = score_pool.tile([128, 1024], F32, tag="T1")
                        T2 = score_pool.tile([128, 1024], F32, tag="T2")
                        # T1 banks 1-2 mix accumulation groups (jt4-c0 has
                        # M=128 bias + M=65 QK; bank 2 holds all five c1
                        # groups). Chain the c1 matmuls serially and skip the
                        # sim group checks there.
                        c1_prev = [None]

                        def mm_c1(out, lhsT, rhs, start, stop):
                            mm = nc.tensor.matmul(
                                out, lhsT, rhs, start=start, stop=stop,
                                skip_group_check=True,
                            )
                            if c1_prev[0] is not None:
                                tile.add_dep_helper(
                                    mm.ins, c1_prev[0].ins, reason="c1 group chain"
                                )
                            c1_prev[0] = mm

                        def kT(jt):
                            kj = JTS[jt]
                            return qk_t[km][qp : qp + 64, N * b + 128 * jt : N * b + 128 * jt + kj]

                        q_c0 = qk_t[qm][qp : qp + 64, N * b : N * b + 512]
                        q_c1 = qk_t[qm][qp : qp + 64, N * b + 512 : N * b + 577]
                        # c0 for jt pairs (0,1), (2,3): biases first, then the
                        # two K=64 QK matmuls back to back so they overlap in
                        # disjoint PE row groups.
                        for jta in (0, 2):
                            for jt in (jta, jta + 1):
                                tl, lc = (T0, T1)[jta // 2], 512 * (jt % 2)
                                nc.tensor.matmul(
                                    tl[:, lc : lc + 512],
                                    ident[:, :],
                                    bias_t[:, 512 * jt : 512 * jt + 512],
                                    start=True,
                                    stop=False,
                                )
                            for jt in (jta, jta + 1):
                                tl, lc = (T0, T1)[jta // 2], 512 * (jt % 2)
                                nc.tensor.matmul(
                                    tl[:, lc : lc + 512],
                                    kT(jt),
                                    q_c0,
                                    start=False,
                                    stop=True,
                                )
                        # jt4 c0 (T2 bank 0, mixed-M group -> chained+skip)
                        mm_c1(T2[:, 0:512], ident[:65, :],
                              bias_t[:65, 2048:2560], True, False)
                        mm_c1(T2[:65, 0:512], kT(4), q_c0,
                              False, True)
                        # c1 regions (i 512..576) in T1 bank 2; bias written
                        # 72 wide (biasp pads cols 65..71 with 0) so the exp
                        # source range is fully initialized.
                        for jt in range(5):
                            kj = JTS[jt]
                            mm_c1(
                                T2[:, 512 + 72 * jt : 512 + 72 * jt + 72],
                                ident[:kj, :],
                                bias_t[:kj, 2560 + 72 * jt : 2560 + 72 * jt + 72],
                                True,
                                False,
                            )
                            mm_c1(
                                T2[:kj, 512 + 72 * jt : 512 + 72 * jt + 65],
                                kT(jt),
                                q_c1,
                                False,
                                True,
                            )
                        E_t = p2.tile([128, ECOLS], BF16, tag="E")
                        nc.scalar.activation(E_t[:, 0:1024], T0[:, :], AF.Exp)
                        nc.scalar.activation(E_t[:, 1024:2048], T1[:, :], AF.Exp)
                        nc.scalar.activation(E_t[:, 2048:2920], T2[:, 0:872], AF.Exp)

                        pv = pv_pool.tile([128, 1024], F32, tag="pv")
                        for ci, (oc0, ocn) in enumerate(((0, 512), (512, 65))):
                            for jt in range(5):
                                kj = JTS[jt]
                                ec = 512 * jt if ci == 0 else 2560 + 72 * jt
                                nc.tensor.matmul(
                                    pv[0:65, oc0 : oc0 + ocn],
                                    v1_t[b][jt][:kj, 65 * h : 65 * h + 65],
                                    E_t[:kj, ec : ec + ocn],
                                    start=(jt == 0),
                                    stop=(jt == 4),
                                )
                        # Evacuate pv quickly (two DVE reads) so the next
                        # (h, b) iteration's PV matmuls can reuse the psum.
                        s_t = p2.tile([65, N], F32, tag="s_t")
                        nc.vector.tensor_copy(s_t[64:65, 0:N], pv[64:65, 0:N])
                        ocp = p2.tile([64, N], BF16, tag="ocp")
                        nc.vector.tensor_copy(ocp[:, :], pv[0:64, 0:N])
                        # HW partition_broadcast reads absolute partition 0:
                        # DMA the denominator row 64 -> 0 within the tile.
                        nc.sync.dma_start(s_t[0:1, :], s_t[64:65, :])
                        rb = p2.tile([64, N], F32, tag="rb")
                        nc.gpsimd.partition_broadcast(rb[:, :], s_t[0:1, :], channels=64)
                        nc.vector.reciprocal_approx_fast(rb[:, :], rb[:, :])
                        if h % 2 == 0:
                            nc.vector.tensor_mul(
                                ot_sb[0:64, R * (h // 2) + N * b : R * (h // 2) + N * (b + 1)],
                                ocp[:, :],
                                rb[:, :],
                            )
                        else:
                            stg = p2.tile([64, N], BF16, tag="stg")
                            nc.vector.tensor_mul(stg[:, :], ocp[:, :], rb[:, :])
                            nc.sync.dma_start(
                                ot_sb[64:128, R * (h // 2) + N * b : R * (h // 2) + N * (b + 1)],
                                stg[:, :],
                            )

            # ---------------- Phase 3: output projection -----------------
            with (
                tc.tile_pool(name="ph3", bufs=6) as p3,
                tc.tile_pool(name="psum3", bufs=8, space="PSUM") as psum3,
            ):
                for m in range(CT):
                    for c0, cn in RCHUNKS:
                        ps = psum3.tile([128, 512], F32, tag="ps3")
                        for kk in range(CT):
                            nc.tensor.matmul(
                                ps[:, :cn],
                                wp_sb[:, C * kk + 128 * m : C * kk + 128 * (m + 1)],
                                ot_sb[:, R * kk + c0 : R * kk + c0 + cn],
                                start=(kk == 0),
                                stop=(kk == CT - 1),
                            )
                        ft = p3.tile([128, 512], F16, tag="ft")
                        nc.scalar.add(ft[:, :cn], ps[:, :cn], pb[:, m : m + 1])
                        nc.sync.dma_start(out_d[:, R * m + c0 : R * m + c0 + cn], ft[:, :cn])


def get_program():
    global _PROGRAM
    if _PROGRAM is None:
        _PROGRAM = build_program()
    return _PROGRAM


def _pack_ctiles(a):
    """[768, X] -> SBUF image [128, 6*X] (c-tile kk at cols kk*X..(kk+1)*X)."""
    rows, cols = a.shape
    assert rows == 768
    return np.ascontiguousarray(
        a.reshape(CT, 128, cols).transpose(1, 0, 2).reshape(128, CT * cols)
    )


def make_host_inputs(x, qkv_w, table, rel_index, proj_w, proj_b):
    bf = ml_dtypes.bfloat16
    f8 = ml_dtypes.float8_e4m3
    x = np.asarray(x, np.float32)
    qkv_w = np.asarray(qkv_w, np.float32)
    table = np.asarray(table, np.float32)
    rel_index = np.asarray(rel_index)
    proj_w = np.asarray(proj_w, np.float32)
    proj_b = np.asarray(proj_b, np.float32)

    qkv_ws = qkv_w.copy()
    qkv_ws[:768] *= 0.125                                    # fold q scale (exact in bf16)
    wt = qkv_ws.T                                            # [768, 2304]
    # qk half m-major: wqk[p, m*768 + kk*128 + cc] = wt[kk*128+p, m*128+cc]
    wqk = np.ascontiguousarray(
        wt[:, :1536].reshape(CT, 128, MT, 128).transpose(1, 2, 0, 3).reshape(128, MT * CT * 128)
    ).astype(bf)
    # v half kk-major: wv[p, kk*768 + vc] = wt[kk*128+p, 1536+vc]
    wv = np.ascontiguousarray(
        wt[:, 1536:].reshape(CT, 128, C).transpose(1, 0, 2).reshape(128, CT * C)
    ).astype(bf)
    wp = _pack_ctiles(proj_w.T).astype(bf)                   # [128, 6*768]
    pb_hi = proj_b.astype(bf)
    pb_lo = (proj_b - pb_hi.astype(np.float32)).astype(bf)
    pbh = np.ascontiguousarray(pb_hi.reshape(CT, 128).T)     # [128, 6]
    pbl = np.ascontiguousarray(pb_lo.reshape(CT, 128).T)
    ident = np.eye(128, dtype=bf)

    # bias, transposed orientation: biasT[h, j, i] = table[rel_index[i, j], h]
    g = table[rel_index.reshape(-1)].reshape(N, N, H)        # [i, j, h]
    bt = g.transpose(2, 1, 0)                                # [h, j, i]
    btp = np.zeros((H, 640, N), np.float32)
    btp[:, :N] = bt
    btp = btp.reshape(H, 5, 128, N)
    c0 = btp[:, :, :, 0:512].transpose(0, 2, 1, 3).reshape(H, 128, 2560)
    c1 = np.zeros((H, 128, 5, 72), np.float32)
    c1[:, :, :, 0:65] = btp[:, :, :, 512:577].transpose(0, 2, 1, 3)
    biasp = np.ascontiguousarray(
        np.concatenate([c0, c1.reshape(H, 128, 360)], axis=2)
    ).astype(f8)                                             # [12, 128, 2920]

    # fp8 bias bytes ride in the bf16 image, two per column
    bias_bf = (
        biasp.transpose(1, 0, 2).reshape(128, H * ECOLS).view(np.uint16).view(bf)
    )                                                        # [128, 12*1460]
    shared = np.concatenate([wqk, wv, wp, ident, pbh, pbl, bias_bf], axis=1)
    in_maps = []
    for c in range(NCORES):
        xT = x[BPC * c : BPC * (c + 1)].reshape(R, C).T      # [768, 2308]
        # batch-major x image: xtb[p, b*XBB + kk*N + j] = xT[kk*128+p, b*N+j]
        xtb = np.ascontiguousarray(
            xT.reshape(CT, 128, BPC, N).transpose(1, 2, 0, 3).reshape(128, BPC * XBB)
        ).astype(bf)
        mega = np.concatenate([xtb, shared], axis=1)
        in_maps.append({"mega": np.ascontiguousarray(mega)})
    return in_maps


def unpack_output(ft):
    """[128, 6*2308] f16 -> [BPC, 577, 768] f32."""
    f = np.asarray(ft, np.float32).reshape(128, CT, R).transpose(1, 0, 2).reshape(C, R)
    return np.ascontiguousarray(f.T).reshape(BPC, N, C)


def kernel(x, qkv_w, table, rel_index, proj_w, proj_b):
    nc = get_program()
    in_maps = make_host_inputs(x, qkv_w, table, rel_index, proj_w, proj_b)
    res = run_bass_kernel_spmd(nc, in_maps, core_ids=list(range(NCORES)))
    out = np.empty((B, N, C), np.float32)
    for c in range(NCORES):
        out[BPC * c : BPC * (c + 1)] = unpack_output(res.results[c]["ftout"])
    return out
